# revision 3
# baseline (speedup 1.0000x reference)
"""Bahdanau additive attention kernel for 8 Trainium2 NeuronCores.

Data-parallel over batch: B=64 -> 8 batches per core. No collectives.

Per-batch math (reference):
  Wa   = dec @ Wa_w.T + Wa_b                       [1, H]
  Ua   = enc @ Ua_w.T + Ua_b                       [Te, H]
  s    = tanh(Ua + Wa) @ Va_w.T  (+ Va_b, dropped: softmax shift-invariant)
  w    = softmax(s)                                 [Te]
  ctx  = w @ enc                                    [1, De]

Device layout (per core, 8 batches):
  big matmul Ua:  out[t, h] = sum_d encT[d, t] * uawT[d, h]    (bf16, fp32 PSUM)
  bias broadcast: K=1 matmul with ones row adds WaPB[b, h] to every t
  tanh:           ScalarE, PSUM -> SBUF
  score:          VectorE tensor_tensor_reduce (x Va bcast, sum over h)
                  -> scores as columns [128t, 8 chunks]
  softmax:        exp on ScalarE (scores bounded, no max subtraction),
                  normalization deferred to the end (unnormalized weights)
  context:        matmul, exp-weight column stationary vs encN [t, d] moving
  scale:          ScalarE copy with scale = 1/sum(exp)
"""

import os
import sys

import numpy as np
import ml_dtypes

for _p in ("/opt/trn_rl_repo",):
    if _p not in sys.path and os.path.isdir(_p):
        sys.path.append(_p)

import concourse.bass as bass
import concourse.tile as tile
import concourse.mybir as mybir
from concourse import bacc
from concourse.bass import ts
from concourse.bass_utils import run_bass_kernel_spmd

B, T, D, H = 64, 1024, 1024, 1024
NCORES = 8
BPC = B // NCORES  # batches per core
P = 128
DC = D // P  # 8 contraction chunks
TC = T // P  # 8 t chunks

BF = mybir.dt.bfloat16
F32 = mybir.dt.float32
AF = mybir.ActivationFunctionType
ALU = mybir.AluOpType


def build_bass():
    nc = bacc.Bacc("TRN2", target_bir_lowering=False, debug=False)

    encT = nc.dram_tensor("encT", [BPC, D, T], BF, kind="ExternalInput")
    encN = nc.dram_tensor("encN", [BPC, T, D], BF, kind="ExternalInput")
    uawT = nc.dram_tensor("uawT", [D, H], BF, kind="ExternalInput")
    wawT = nc.dram_tensor("wawT", [D, H], BF, kind="ExternalInput")
    decT = nc.dram_tensor("decT", [D, BPC], BF, kind="ExternalInput")
    bsum = nc.dram_tensor("bsum", [1, H], BF, kind="ExternalInput")
    vabc = nc.dram_tensor("vabc", [P, H], F32, kind="ExternalInput")
    out = nc.dram_tensor("out", [BPC, D], F32, kind="ExternalOutput")

    with tile.TileContext(nc) as tc:
        with (
            tc.tile_pool(name="const", bufs=1) as cpool,
            tc.tile_pool(name="enc", bufs=2) as epool,
            tc.tile_pool(name="work", bufs=3) as wpool,
            tc.tile_pool(name="pu", bufs=4, space="PSUM") as pupool,
            tc.tile_pool(name="pc", bufs=2, space="PSUM") as pcpool,
        ):
            # resident weights / constants
            UW = cpool.tile([P, DC, H], BF, tag="UW")
            nc.sync.dma_start(UW[:], uawT.ap().rearrange("(dc p) h -> p dc h", p=P))
            WW = cpool.tile([P, DC, H], BF, tag="WW")
            nc.sync.dma_start(WW[:], wawT.ap().rearrange("(dc p) h -> p dc h", p=P))
            DT = cpool.tile([P, DC, BPC], BF, tag="DT")
            nc.sync.dma_start(DT[:], decT.ap().rearrange("(dc p) b -> p dc b", p=P))
            BS = cpool.tile([1, H], BF, tag="BS")
            nc.sync.dma_start(BS[:], bsum.ap())
            VAB = cpool.tile([P, H], F32, tag="VAB")
            nc.sync.dma_start(VAB[:], vabc.ap())

            ones_r = cpool.tile([1, P], BF, tag="ones_r")
            nc.vector.memset(ones_r[:], 1.0)
            ones_c = cpool.tile([P, 1], BF, tag="ones_c")
            nc.vector.memset(ones_c[:], 1.0)

            for b in range(BPC):
                EB = epool.tile([P, DC, T], BF, tag="EB")
                nc.sync.dma_start(
                    EB[:], encT.ap()[b].rearrange("(dc p) t -> p dc t", p=P)
                )
                NB = epool.tile([P, TC, D], BF, tag="NB")
                nc.sync.dma_start(
                    NB[:], encN.ap()[b].rearrange("(tc p) d -> p tc d", p=P)
                )

                # WaPB_b[h] = dec_b @ Wa_w.T + (Wa_b + Ua_b)   -> [1, H] bf16
                WaPB = wpool.tile([1, H], BF, tag="WaPB")
                for hh in range(2):
                    pw = pcpool.tile([1, 512], F32, tag="pc")
                    for dc in range(DC):
                        nc.tensor.matmul(
                            pw[:],
                            DT[:, dc, b : b + 1],
                            WW[:, dc, ts(hh, 512)],
                            start=(dc == 0),
                            stop=False,
                        )
                    nc.tensor.matmul(
                        pw[:],
                        ones_r[:, 0:1],
                        BS[:, ts(hh, 512)],
                        start=False,
                        stop=True,
                    )
                    nc.vector.tensor_copy(WaPB[:, ts(hh, 512)], pw[:])

                SC = wpool.tile([P, TC], F32, tag="SC")
                for tci in range(TC):
                    pu0 = pupool.tile([P, 512], F32, tag="pu")
                    pu1 = pupool.tile([P, 512], F32, tag="pu")
                    for dc in range(DC):
                        lh = EB[:, dc, ts(tci, P)]
                        nc.tensor.matmul(
                            pu0[:], lh, UW[:, dc, 0:512], start=(dc == 0), stop=False
                        )
                        nc.tensor.matmul(
                            pu1[:], lh, UW[:, dc, 512:1024], start=(dc == 0), stop=False
                        )
                    # += WaPB broadcast along t partitions (K=1 ones matmul)
                    nc.tensor.matmul(
                        pu0[:], ones_r[:], WaPB[:, 0:512], start=False, stop=True
                    )
                    nc.tensor.matmul(
                        pu1[:], ones_r[:], WaPB[:, 512:1024], start=False, stop=True
                    )
                    TH = wpool.tile([P, H], F32, tag="TH")
                    nc.scalar.activation(TH[:, 0:512], pu0[:], AF.Tanh)
                    nc.scalar.activation(TH[:, 512:1024], pu1[:], AF.Tanh)
                    TMP = wpool.tile([P, H], F32, tag="TMP")
                    nc.vector.tensor_tensor(TMP[:], TH[:], VAB[:], ALU.mult)
                    nc.vector.tensor_reduce(
                        SC[:, tci : tci + 1],
                        TMP[:],
                        axis=mybir.AxisListType.X,
                        op=ALU.add,
                    )

                # unnormalized softmax weights, bf16 columns [128t, TC]
                EW = wpool.tile([P, TC], BF, tag="EW")
                nc.scalar.activation(EW[:], SC[:], AF.Exp)
                psum_s = pcpool.tile([1, TC], F32, tag="pc")
                nc.tensor.matmul(psum_s[:], ones_c[:], EW[:], start=True, stop=True)
                TOT = wpool.tile([1, 1], F32, tag="TOT")
                nc.vector.tensor_reduce(
                    TOT[:], psum_s[:], axis=mybir.AxisListType.X, op=ALU.add
                )
                INV = wpool.tile([1, 1], F32, tag="INV")
                nc.vector.reciprocal(INV[:], TOT[:])

                OUTb = wpool.tile([1, D], F32, tag="OUTb")
                for dh in range(2):
                    pc = pcpool.tile([1, 512], F32, tag="pc")
                    for tci in range(TC):
                        nc.tensor.matmul(
                            pc[:],
                            EW[:, tci : tci + 1],
                            NB[:, tci, ts(dh, 512)],
                            start=(tci == 0),
                            stop=(tci == TC - 1),
                        )
                    nc.scalar.activation(
                        OUTb[:, ts(dh, 512)], pc[:], AF.Copy, scale=INV[:]
                    )
                nc.sync.dma_start(out.ap()[b : b + 1, :], OUTb[:])

    nc.finalize()
    return nc


_NC = None


def _get_nc():
    global _NC
    if _NC is None:
        _NC = build_bass()
    return _NC


LAST_RESULTS = None


def kernel(**inputs) -> np.ndarray:
    enc = np.asarray(inputs["encoder_outputs"], dtype=np.float32)  # [B, T, D]
    dec = np.asarray(inputs["decoder_outputs"], dtype=np.float32)[:, 0, :]  # [B, D]
    Wa_w = np.asarray(inputs["Wa_w"], dtype=np.float32)
    Wa_b = np.asarray(inputs["Wa_b"], dtype=np.float32)
    Ua_w = np.asarray(inputs["Ua_w"], dtype=np.float32)
    Ua_b = np.asarray(inputs["Ua_b"], dtype=np.float32)
    Va_w = np.asarray(inputs["Va_w"], dtype=np.float32)
    # Va_b dropped: softmax(s + c) == softmax(s)

    bf16 = ml_dtypes.bfloat16
    enc_bf = enc.astype(bf16)  # [B, T, D]
    encN_all = enc_bf.reshape(NCORES, BPC, T, D)
    encT_all = np.ascontiguousarray(enc_bf.transpose(0, 2, 1)).reshape(
        NCORES, BPC, D, T
    )
    decT_all = np.ascontiguousarray(
        dec.reshape(NCORES, BPC, D).transpose(0, 2, 1)
    ).astype(bf16)  # [NCORES, D, BPC]
    uawT = np.ascontiguousarray(Ua_w.T).astype(bf16)
    wawT = np.ascontiguousarray(Wa_w.T).astype(bf16)
    bsum = (Wa_b + Ua_b).reshape(1, H).astype(bf16)
    vabc = np.ascontiguousarray(np.broadcast_to(Va_w.reshape(1, H), (P, H))).astype(
        np.float32
    )

    in_maps = [
        {
            "encT": np.ascontiguousarray(encT_all[c]),
            "encN": np.ascontiguousarray(encN_all[c]),
            "uawT": uawT,
            "wawT": wawT,
            "decT": np.ascontiguousarray(decT_all[c]),
            "bsum": bsum,
            "vabc": vabc,
        }
        for c in range(NCORES)
    ]

    nc = _get_nc()
    trace = bool(int(os.environ.get("KERNEL_TRACE", "0")))
    res = run_bass_kernel_spmd(
        nc, in_maps, core_ids=list(range(NCORES)), trace=trace
    )
    global LAST_RESULTS
    LAST_RESULTS = res

    outs = [res.results[c]["out"] for c in range(NCORES)]
    full = np.concatenate(outs, axis=0).reshape(B, 1, D).astype(np.float32)
    return full


# revision 10
# speedup vs baseline: 1.3657x; 1.3657x over previous
"""Bahdanau additive attention kernel for 8 Trainium2 NeuronCores.

Data-parallel over batch: B=64 -> 8 batches per core. No collectives.

Per-batch math (reference):
  Wa   = dec @ Wa_w.T + Wa_b                       [1, H]
  Ua   = enc @ Ua_w.T + Ua_b                       [Te, H]
  s    = tanh(Ua + Wa) @ Va_w.T  (+ Va_b, dropped: softmax shift-invariant)
  w    = softmax(s)                                 [Te]
  ctx  = w @ enc                                    [1, De]

Device layout (per core, 8 batches):
  big matmul Ua:  out[t, h] = sum_d encT[d, t] * uawT[d, h]    (bf16, fp32 PSUM)
  bias:           VectorE add of WaPB broadcast (built once per b via ones mm)
  tanh:           ScalarE, SBUF -> SBUF bf16
  score:          VectorE mult by Va bcast (bf16 2x mode) + reduce over h
                  -> scores as columns [128t, 8 chunks]
  softmax:        exp on ScalarE (scores bounded, no max subtraction),
                  normalization deferred to the end (unnormalized weights)
  context:        matmul, exp-weight column stationary vs encN [t, d] moving;
                  software-pipelined one batch behind the score stage
  scale:          ScalarE copy with scale = 1/sum(exp)
"""

import os
import sys

import numpy as np
import ml_dtypes

for _p in ("/opt/trn_rl_repo",):
    if _p not in sys.path and os.path.isdir(_p):
        sys.path.append(_p)

import concourse.bass as bass
import concourse.tile as tile
import concourse.mybir as mybir
from concourse import bacc
from concourse.bass import ts
from concourse.bass_utils import run_bass_kernel_spmd

B, T, D, H = 64, 1024, 1024, 1024
NCORES = 8
BPC = B // NCORES  # batches per core
P = 128
DC = D // P  # 8 contraction chunks
TC = T // P  # 8 t chunks

BF = mybir.dt.bfloat16
F32 = mybir.dt.float32
AF = mybir.ActivationFunctionType
ALU = mybir.AluOpType


def build_bass(bias_on: str = "vector", score_bf16: bool = True, pipelined: bool = True):
    nc = bacc.Bacc("TRN2", target_bir_lowering=False, debug=False)

    va_dt = BF if score_bf16 else F32
    th_dt = BF if score_bf16 else F32

    encT = nc.dram_tensor("encT", [BPC, D, T], BF, kind="ExternalInput")
    encN = nc.dram_tensor("encN", [BPC, T, D], BF, kind="ExternalInput")
    uawT = nc.dram_tensor("uawT", [D, H], BF, kind="ExternalInput")
    wawT = nc.dram_tensor("wawT", [D, H], BF, kind="ExternalInput")
    decT = nc.dram_tensor("decT", [D, BPC], BF, kind="ExternalInput")
    bsum = nc.dram_tensor("bsum", [1, H], BF, kind="ExternalInput")
    vabc = nc.dram_tensor("vabc", [P, H], va_dt, kind="ExternalInput")
    out = nc.dram_tensor("out", [BPC, D], F32, kind="ExternalOutput")

    with tile.TileContext(nc) as tc:
        with (
            tc.tile_pool(name="const", bufs=1) as cpool,
            tc.tile_pool(name="enc", bufs=2) as epool,
            tc.tile_pool(name="work", bufs=3) as wpool,
            tc.tile_pool(name="pu", bufs=4, space="PSUM") as pupool,
            tc.tile_pool(name="pc", bufs=2, space="PSUM") as pcpool,
        ):
            # resident weights / constants
            UW = cpool.tile([P, DC, H], BF, tag="UW")
            nc.sync.dma_start(UW[:], uawT.ap().rearrange("(dc p) h -> p dc h", p=P))
            WW = cpool.tile([P, DC, H], BF, tag="WW")
            nc.sync.dma_start(WW[:], wawT.ap().rearrange("(dc p) h -> p dc h", p=P))
            DT = cpool.tile([P, DC, BPC], BF, tag="DT")
            nc.sync.dma_start(DT[:], decT.ap().rearrange("(dc p) b -> p dc b", p=P))
            BS = cpool.tile([1, H], BF, tag="BS")
            nc.sync.dma_start(BS[:], bsum.ap())
            VAB = cpool.tile([P, H], va_dt, tag="VAB")
            nc.sync.dma_start(VAB[:], vabc.ap())

            ones_r = cpool.tile([1, P], BF, tag="ones_r")
            nc.vector.memset(ones_r[:], 1.0)
            ones_c = cpool.tile([P, 1], BF, tag="ones_c")
            nc.vector.memset(ones_c[:], 1.0)

            # WaPB[b, h] = dec_b @ Wa_w.T + (Wa_b + Ua_b), all batches at once,
            # then flattened to one partition so per-b rows are base-0 matmul rhs.
            WaPBs = cpool.tile([BPC, H], BF, tag="WaPBs")
            for hh in range(2):
                pw = pcpool.tile([BPC, 512], F32, tag="pc")
                for dc in range(DC):
                    nc.tensor.matmul(
                        pw[:],
                        DT[:, dc, :],
                        WW[:, dc, ts(hh, 512)],
                        start=(dc == 0),
                        stop=False,
                    )
                nc.tensor.matmul(
                    pw[:],
                    ones_r[:, 0:BPC],
                    BS[:, ts(hh, 512)],
                    start=False,
                    stop=True,
                )
                nc.vector.tensor_copy(WaPBs[:, ts(hh, 512)], pw[:])
            WaPBrow = cpool.tile([1, BPC * H], BF, tag="WaPBrow")
            for b in range(BPC):
                nc.sync.dma_start(WaPBrow[:, b * H : (b + 1) * H], WaPBs[b : b + 1, :])

            def scores_stage(b):
                EB = epool.tile([P, DC, T], BF, tag="EB")
                nc.sync.dma_start(
                    EB[:], encT.ap()[b].rearrange("(dc p) t -> p dc t", p=P)
                )
                NB = epool.tile([P, TC, D], BF, tag="NB")
                nc.sync.dma_start(
                    NB[:], encN.ap()[b].rearrange("(tc p) d -> p tc d", p=P)
                )

                WaPB = WaPBrow[:, b * H : (b + 1) * H]
                if bias_on == "vector":
                    # broadcast WaPB to 128 partitions once per b (ones matmul)
                    WB = wpool.tile([P, H], F32, tag="WB")
                    for hh in range(2):
                        pb = pcpool.tile([P, 512], F32, tag="pb")
                        nc.tensor.matmul(
                            pb[:],
                            ones_r[:],
                            WaPB[:, ts(hh, 512)],
                            start=True,
                            stop=True,
                        )
                        nc.vector.tensor_copy(WB[:, ts(hh, 512)], pb[:])
                SC = wpool.tile([P, TC], F32, tag="SC")
                for tci in range(TC):
                    pu0 = pupool.tile([P, 512], F32, tag="pu")
                    pu1 = pupool.tile([P, 512], F32, tag="pu")
                    last = bias_on != "tensor"
                    for dc in range(DC):
                        lh = EB[:, dc, ts(tci, P)]
                        nc.tensor.matmul(
                            pu0[:],
                            lh,
                            UW[:, dc, 0:512],
                            start=(dc == 0),
                            stop=(last and dc == DC - 1),
                        )
                        nc.tensor.matmul(
                            pu1[:],
                            lh,
                            UW[:, dc, 512:1024],
                            start=(dc == 0),
                            stop=(last and dc == DC - 1),
                        )
                    TH = wpool.tile([P, H], th_dt, tag="TH")
                    if bias_on == "tensor":
                        # += WaPB broadcast along t partitions (K=1 ones matmul)
                        nc.tensor.matmul(
                            pu0[:], ones_r[:], WaPB[:, 0:512], start=False, stop=True
                        )
                        nc.tensor.matmul(
                            pu1[:], ones_r[:], WaPB[:, 512:1024], start=False, stop=True
                        )
                        nc.scalar.activation(TH[:, 0:512], pu0[:], AF.Tanh)
                        nc.scalar.activation(TH[:, 512:1024], pu1[:], AF.Tanh)
                    else:
                        T1 = wpool.tile([P, H], F32, tag="T1")
                        nc.vector.tensor_tensor(
                            T1[:, 0:512], pu0[:], WB[:, 0:512], ALU.add
                        )
                        nc.vector.tensor_tensor(
                            T1[:, 512:1024], pu1[:], WB[:, 512:1024], ALU.add
                        )
                        nc.scalar.activation(TH[:, 0:512], T1[:, 0:512], AF.Tanh)
                        nc.scalar.activation(TH[:, 512:1024], T1[:, 512:1024], AF.Tanh)
                    TMP = wpool.tile([P, H], th_dt, tag="TMP")
                    nc.vector.tensor_tensor(TMP[:], TH[:], VAB[:], ALU.mult)
                    nc.vector.tensor_reduce(
                        SC[:, tci : tci + 1],
                        TMP[:],
                        axis=mybir.AxisListType.X,
                        op=ALU.add,
                    )
                return SC, NB

            def ctx_stage(b, SC, NB):
                # unnormalized softmax weights, bf16 columns [128t, TC]
                EW = wpool.tile([P, TC], BF, tag="EW")
                nc.scalar.activation(EW[:], SC[:], AF.Exp)
                psum_s = pcpool.tile([1, TC], F32, tag="pc")
                nc.tensor.matmul(psum_s[:], ones_c[:], EW[:], start=True, stop=True)
                TOT = wpool.tile([1, 1], F32, tag="TOT")
                nc.vector.tensor_reduce(
                    TOT[:], psum_s[:], axis=mybir.AxisListType.X, op=ALU.add
                )
                INV = wpool.tile([1, 1], F32, tag="INV")
                nc.vector.reciprocal(INV[:], TOT[:])

                OUTb = wpool.tile([1, D], F32, tag="OUTb")
                for dh in range(2):
                    pc = pcpool.tile([1, 512], F32, tag="pc")
                    for tci in range(TC):
                        nc.tensor.matmul(
                            pc[:],
                            EW[:, tci : tci + 1],
                            NB[:, tci, ts(dh, 512)],
                            start=(tci == 0),
                            stop=(tci == TC - 1),
                        )
                    nc.scalar.activation(
                        OUTb[:, ts(dh, 512)], pc[:], AF.Copy, scale=INV[:]
                    )
                nc.sync.dma_start(out.ap()[b : b + 1, :], OUTb[:])

            if pipelined:
                prev = None
                for b in range(BPC):
                    cur = scores_stage(b)
                    if prev is not None:
                        ctx_stage(b - 1, *prev)
                    prev = cur
                ctx_stage(BPC - 1, *prev)
            else:
                for b in range(BPC):
                    SC, NB = scores_stage(b)
                    ctx_stage(b, SC, NB)

    nc.finalize()
    return nc


_NC = None


def _get_nc():
    global _NC
    if _NC is None:
        _NC = build_bass()
    return _NC


LAST_RESULTS = None


def kernel(**inputs) -> np.ndarray:
    enc = np.asarray(inputs["encoder_outputs"], dtype=np.float32)  # [B, T, D]
    dec = np.asarray(inputs["decoder_outputs"], dtype=np.float32)[:, 0, :]  # [B, D]
    Wa_w = np.asarray(inputs["Wa_w"], dtype=np.float32)
    Wa_b = np.asarray(inputs["Wa_b"], dtype=np.float32)
    Ua_w = np.asarray(inputs["Ua_w"], dtype=np.float32)
    Ua_b = np.asarray(inputs["Ua_b"], dtype=np.float32)
    Va_w = np.asarray(inputs["Va_w"], dtype=np.float32)
    # Va_b dropped: softmax(s + c) == softmax(s)

    bf16 = ml_dtypes.bfloat16
    enc_bf = enc.astype(bf16)  # [B, T, D]
    encN_all = enc_bf.reshape(NCORES, BPC, T, D)
    encT_all = np.ascontiguousarray(enc_bf.transpose(0, 2, 1)).reshape(
        NCORES, BPC, D, T
    )
    decT_all = np.ascontiguousarray(
        dec.reshape(NCORES, BPC, D).transpose(0, 2, 1)
    ).astype(bf16)  # [NCORES, D, BPC]
    uawT = np.ascontiguousarray(Ua_w.T).astype(bf16)
    wawT = np.ascontiguousarray(Wa_w.T).astype(bf16)
    bsum = (Wa_b + Ua_b).reshape(1, H).astype(bf16)
    vabc = np.ascontiguousarray(np.broadcast_to(Va_w.reshape(1, H), (P, H))).astype(
        bf16
    )

    in_maps = [
        {
            "encT": np.ascontiguousarray(encT_all[c]),
            "encN": np.ascontiguousarray(encN_all[c]),
            "uawT": uawT,
            "wawT": wawT,
            "decT": np.ascontiguousarray(decT_all[c]),
            "bsum": bsum,
            "vabc": vabc,
        }
        for c in range(NCORES)
    ]

    nc = _get_nc()
    trace = bool(int(os.environ.get("KERNEL_TRACE", "0")))
    res = run_bass_kernel_spmd(nc, in_maps, core_ids=list(range(NCORES)), trace=trace)
    global LAST_RESULTS
    LAST_RESULTS = res

    outs = [res.results[c]["out"] for c in range(NCORES)]
    full = np.concatenate(outs, axis=0).reshape(B, 1, D).astype(np.float32)
    return full


# revision 12
# speedup vs baseline: 1.5411x; 1.1284x over previous
"""Bahdanau additive attention kernel for 8 Trainium2 NeuronCores.

Data-parallel over batch: B=64 -> 8 batches per core. No collectives.

Per-batch math (reference):
  Wa   = dec @ Wa_w.T + Wa_b                       [1, H]
  Ua   = enc @ Ua_w.T + Ua_b                       [Te, H]
  s    = tanh(Ua + Wa) @ Va_w.T  (+ Va_b, dropped: softmax shift-invariant)
  w    = softmax(s)                                 [Te]
  ctx  = w @ enc                                    [1, De]

Device layout (per core, 8 batches):
  big matmul Ua:  out[t, h] = sum_d encT[d, t] * uawT[d, h]    (bf16, fp32 PSUM)
  bias:           VectorE add of WaPB broadcast (built once per b via ones mm)
  tanh:           ScalarE, SBUF -> SBUF bf16
  score:          VectorE mult by Va bcast (bf16 2x mode) + reduce over h
                  -> scores as columns [128t, 8 chunks]
  softmax:        exp on ScalarE (scores bounded, no max subtraction),
                  normalization deferred to the end (unnormalized weights)
  context:        matmul, exp-weight column stationary vs encN [t, d] moving;
                  software-pipelined one batch behind the score stage
  scale:          ScalarE copy with scale = 1/sum(exp)
"""

import os
import sys

import numpy as np
import ml_dtypes

for _p in ("/opt/trn_rl_repo",):
    if _p not in sys.path and os.path.isdir(_p):
        sys.path.append(_p)

import concourse.bass as bass
import concourse.tile as tile
import concourse.mybir as mybir
from concourse import bacc
from concourse.bass import ts
from concourse.bass_utils import run_bass_kernel_spmd

B, T, D, H = 64, 1024, 1024, 1024
NCORES = 8
BPC = B // NCORES  # batches per core
P = 128
DC = D // P  # 8 contraction chunks
TC = T // P  # 8 t chunks

BF = mybir.dt.bfloat16
F32 = mybir.dt.float32
AF = mybir.ActivationFunctionType
ALU = mybir.AluOpType


def build_bass(
    bias_on: str = "vector",
    score_bf16: bool = True,
    pipelined: bool = True,
    enc_bufs: int = 2,
    work_bufs: int = 3,
    pu_bufs: int = 4,
    pc_bufs: int = 2,
):
    nc = bacc.Bacc("TRN2", target_bir_lowering=False, debug=False)

    va_dt = BF if score_bf16 else F32
    th_dt = BF if score_bf16 else F32

    encT = nc.dram_tensor("encT", [BPC, D, T], BF, kind="ExternalInput")
    encN = nc.dram_tensor("encN", [BPC, T, D], BF, kind="ExternalInput")
    uawT = nc.dram_tensor("uawT", [D, H], BF, kind="ExternalInput")
    wawT = nc.dram_tensor("wawT", [D, H], BF, kind="ExternalInput")
    decT = nc.dram_tensor("decT", [D, BPC], BF, kind="ExternalInput")
    bsum = nc.dram_tensor("bsum", [1, H], BF, kind="ExternalInput")
    vabc = nc.dram_tensor("vabc", [P, H], va_dt, kind="ExternalInput")
    out = nc.dram_tensor("out", [BPC, D], F32, kind="ExternalOutput")

    with tile.TileContext(nc) as tc:
        with (
            tc.tile_pool(name="const", bufs=1) as cpool,
            tc.tile_pool(name="enc", bufs=enc_bufs) as epool,
            tc.tile_pool(name="work", bufs=work_bufs) as wpool,
            tc.tile_pool(name="pu", bufs=pu_bufs, space="PSUM") as pupool,
            tc.tile_pool(name="pc", bufs=pc_bufs, space="PSUM") as pcpool,
        ):
            # resident weights / constants
            UW = cpool.tile([P, DC, H], BF, tag="UW")
            nc.sync.dma_start(UW[:], uawT.ap().rearrange("(dc p) h -> p dc h", p=P))
            WW = cpool.tile([P, DC, H], BF, tag="WW")
            nc.sync.dma_start(WW[:], wawT.ap().rearrange("(dc p) h -> p dc h", p=P))
            DT = cpool.tile([P, DC, BPC], BF, tag="DT")
            nc.sync.dma_start(DT[:], decT.ap().rearrange("(dc p) b -> p dc b", p=P))
            BS = cpool.tile([1, H], BF, tag="BS")
            nc.sync.dma_start(BS[:], bsum.ap())
            VAB = cpool.tile([P, H], va_dt, tag="VAB")
            nc.sync.dma_start(VAB[:], vabc.ap())

            ones_r = cpool.tile([1, P], BF, tag="ones_r")
            nc.vector.memset(ones_r[:], 1.0)
            ones_c = cpool.tile([P, 1], BF, tag="ones_c")
            nc.vector.memset(ones_c[:], 1.0)

            # WaPB[b, h] = dec_b @ Wa_w.T + (Wa_b + Ua_b), all batches at once,
            # then flattened to one partition so per-b rows are base-0 matmul rhs.
            WaPBs = cpool.tile([BPC, H], BF, tag="WaPBs")
            for hh in range(2):
                pw = pcpool.tile([BPC, 512], F32, tag="pc")
                for dc in range(DC):
                    nc.tensor.matmul(
                        pw[:],
                        DT[:, dc, :],
                        WW[:, dc, ts(hh, 512)],
                        start=(dc == 0),
                        stop=False,
                    )
                nc.tensor.matmul(
                    pw[:],
                    ones_r[:, 0:BPC],
                    BS[:, ts(hh, 512)],
                    start=False,
                    stop=True,
                )
                nc.vector.tensor_copy(WaPBs[:, ts(hh, 512)], pw[:])
            WaPBrow = cpool.tile([1, BPC * H], BF, tag="WaPBrow")
            for b in range(BPC):
                nc.sync.dma_start(WaPBrow[:, b * H : (b + 1) * H], WaPBs[b : b + 1, :])

            def scores_stage(b):
                EB = epool.tile([P, DC, T], BF, tag="EB")
                nc.sync.dma_start(
                    EB[:], encT.ap()[b].rearrange("(dc p) t -> p dc t", p=P)
                )
                NB = epool.tile([P, TC, D], BF, tag="NB")
                nc.sync.dma_start(
                    NB[:], encN.ap()[b].rearrange("(tc p) d -> p tc d", p=P)
                )

                WaPB = WaPBrow[:, b * H : (b + 1) * H]
                if bias_on == "vector":
                    # broadcast WaPB to 128 partitions once per b (ones matmul)
                    WB = wpool.tile([P, H], F32, tag="WB")
                    for hh in range(2):
                        pb = pcpool.tile([P, 512], F32, tag="pb")
                        nc.tensor.matmul(
                            pb[:],
                            ones_r[:],
                            WaPB[:, ts(hh, 512)],
                            start=True,
                            stop=True,
                        )
                        nc.vector.tensor_copy(WB[:, ts(hh, 512)], pb[:])
                SC = wpool.tile([P, TC], F32, tag="SC")
                for tci in range(TC):
                    pu0 = pupool.tile([P, 512], F32, tag="pu")
                    pu1 = pupool.tile([P, 512], F32, tag="pu")
                    last = bias_on != "tensor"
                    for dc in range(DC):
                        lh = EB[:, dc, ts(tci, P)]
                        nc.tensor.matmul(
                            pu0[:],
                            lh,
                            UW[:, dc, 0:512],
                            start=(dc == 0),
                            stop=(last and dc == DC - 1),
                        )
                        nc.tensor.matmul(
                            pu1[:],
                            lh,
                            UW[:, dc, 512:1024],
                            start=(dc == 0),
                            stop=(last and dc == DC - 1),
                        )
                    TH = wpool.tile([P, H], th_dt, tag="TH")
                    if bias_on == "tensor":
                        # += WaPB broadcast along t partitions (K=1 ones matmul)
                        nc.tensor.matmul(
                            pu0[:], ones_r[:], WaPB[:, 0:512], start=False, stop=True
                        )
                        nc.tensor.matmul(
                            pu1[:], ones_r[:], WaPB[:, 512:1024], start=False, stop=True
                        )
                        nc.scalar.activation(TH[:, 0:512], pu0[:], AF.Tanh)
                        nc.scalar.activation(TH[:, 512:1024], pu1[:], AF.Tanh)
                    else:
                        T1 = wpool.tile([P, H], F32, tag="T1")
                        nc.vector.tensor_tensor(
                            T1[:, 0:512], pu0[:], WB[:, 0:512], ALU.add
                        )
                        nc.vector.tensor_tensor(
                            T1[:, 512:1024], pu1[:], WB[:, 512:1024], ALU.add
                        )
                        nc.scalar.activation(TH[:, 0:512], T1[:, 0:512], AF.Tanh)
                        nc.scalar.activation(TH[:, 512:1024], T1[:, 512:1024], AF.Tanh)
                    TMP = wpool.tile([P, H], th_dt, tag="TMP")
                    nc.vector.tensor_tensor(TMP[:], TH[:], VAB[:], ALU.mult)
                    nc.vector.tensor_reduce(
                        SC[:, tci : tci + 1],
                        TMP[:],
                        axis=mybir.AxisListType.X,
                        op=ALU.add,
                    )
                return SC, NB

            def ctx_stage(b, SC, NB):
                # unnormalized softmax weights, bf16 columns [128t, TC]
                EW = wpool.tile([P, TC], BF, tag="EW")
                nc.scalar.activation(EW[:], SC[:], AF.Exp)
                psum_s = pcpool.tile([1, TC], F32, tag="pc")
                nc.tensor.matmul(psum_s[:], ones_c[:], EW[:], start=True, stop=True)
                TOT = wpool.tile([1, 1], F32, tag="TOT")
                nc.vector.tensor_reduce(
                    TOT[:], psum_s[:], axis=mybir.AxisListType.X, op=ALU.add
                )
                INV = wpool.tile([1, 1], F32, tag="INV")
                nc.vector.reciprocal(INV[:], TOT[:])

                OUTb = wpool.tile([1, D], F32, tag="OUTb")
                for dh in range(2):
                    pc = pcpool.tile([1, 512], F32, tag="pc")
                    for tci in range(TC):
                        nc.tensor.matmul(
                            pc[:],
                            EW[:, tci : tci + 1],
                            NB[:, tci, ts(dh, 512)],
                            start=(tci == 0),
                            stop=(tci == TC - 1),
                        )
                    nc.scalar.activation(
                        OUTb[:, ts(dh, 512)], pc[:], AF.Copy, scale=INV[:]
                    )
                nc.sync.dma_start(out.ap()[b : b + 1, :], OUTb[:])

            if pipelined:
                prev = None
                for b in range(BPC):
                    cur = scores_stage(b)
                    if prev is not None:
                        ctx_stage(b - 1, *prev)
                    prev = cur
                ctx_stage(BPC - 1, *prev)
            else:
                for b in range(BPC):
                    SC, NB = scores_stage(b)
                    ctx_stage(b, SC, NB)

    nc.finalize()
    return nc


_NC = None


def _get_nc():
    global _NC
    if _NC is None:
        _NC = build_bass()
    return _NC


LAST_RESULTS = None


def kernel(**inputs) -> np.ndarray:
    enc = np.asarray(inputs["encoder_outputs"], dtype=np.float32)  # [B, T, D]
    dec = np.asarray(inputs["decoder_outputs"], dtype=np.float32)[:, 0, :]  # [B, D]
    Wa_w = np.asarray(inputs["Wa_w"], dtype=np.float32)
    Wa_b = np.asarray(inputs["Wa_b"], dtype=np.float32)
    Ua_w = np.asarray(inputs["Ua_w"], dtype=np.float32)
    Ua_b = np.asarray(inputs["Ua_b"], dtype=np.float32)
    Va_w = np.asarray(inputs["Va_w"], dtype=np.float32)
    # Va_b dropped: softmax(s + c) == softmax(s)

    bf16 = ml_dtypes.bfloat16
    enc_bf = enc.astype(bf16)  # [B, T, D]
    encN_all = enc_bf.reshape(NCORES, BPC, T, D)
    encT_all = np.ascontiguousarray(enc_bf.transpose(0, 2, 1)).reshape(
        NCORES, BPC, D, T
    )
    decT_all = np.ascontiguousarray(
        dec.reshape(NCORES, BPC, D).transpose(0, 2, 1)
    ).astype(bf16)  # [NCORES, D, BPC]
    uawT = np.ascontiguousarray(Ua_w.T).astype(bf16)
    wawT = np.ascontiguousarray(Wa_w.T).astype(bf16)
    bsum = (Wa_b + Ua_b).reshape(1, H).astype(bf16)
    vabc = np.ascontiguousarray(np.broadcast_to(Va_w.reshape(1, H), (P, H))).astype(
        bf16
    )

    in_maps = [
        {
            "encT": np.ascontiguousarray(encT_all[c]),
            "encN": np.ascontiguousarray(encN_all[c]),
            "uawT": uawT,
            "wawT": wawT,
            "decT": np.ascontiguousarray(decT_all[c]),
            "bsum": bsum,
            "vabc": vabc,
        }
        for c in range(NCORES)
    ]

    nc = _get_nc()
    trace = bool(int(os.environ.get("KERNEL_TRACE", "0")))
    res = run_bass_kernel_spmd(nc, in_maps, core_ids=list(range(NCORES)), trace=trace)
    global LAST_RESULTS
    LAST_RESULTS = res

    outs = [res.results[c]["out"] for c in range(NCORES)]
    full = np.concatenate(outs, axis=0).reshape(B, 1, D).astype(np.float32)
    return full


# revision 17
# speedup vs baseline: 3.5903x; 2.3297x over previous
"""Bahdanau additive attention kernel for 8 Trainium2 NeuronCores.

Data-parallel over batch: B=64 -> 8 batches per core. No collectives.

Per-batch math (reference):
  Wa   = dec @ Wa_w.T + Wa_b                       [1, H]
  Ua   = enc @ Ua_w.T + Ua_b                       [Te, H]
  s    = tanh(Ua + Wa) @ Va_w.T  (+ Va_b, dropped: softmax shift-invariant)
  w    = softmax(s)                                 [Te]
  ctx  = w @ enc                                    [1, De]

Device layout (per core, 8 batches):
  big matmul Ua:  out[t, h] = sum_d encT[d, t] * uawT[d, h]    (bf16, fp32 PSUM)
  bias:           VectorE add of WaPB broadcast (built once per b via ones mm)
  tanh:           ScalarE, SBUF -> SBUF bf16
  score:          VectorE mult by Va bcast (bf16 2x mode) + reduce over h
                  -> scores as columns [128t, 8 chunks]
  softmax:        exp on ScalarE (scores bounded, no max subtraction),
                  normalization deferred to the end (unnormalized weights)
  context:        matmul, exp-weight column stationary vs encN [t, d] moving;
                  software-pipelined one batch behind the score stage
  scale:          ScalarE copy with scale = 1/sum(exp)
"""

import os
import sys

import numpy as np
import ml_dtypes

for _p in ("/opt/trn_rl_repo",):
    if _p not in sys.path and os.path.isdir(_p):
        sys.path.append(_p)

import concourse.bass as bass
import concourse.tile as tile
import concourse.mybir as mybir
from concourse import bacc
from concourse.bass import ts
from concourse.bass_utils import run_bass_kernel_spmd

B, T, D, H = 64, 1024, 1024, 1024
NCORES = 8
BPC = B // NCORES  # batches per core
P = 128
DC = D // P  # 8 contraction chunks
TC = T // P  # 8 t chunks

BF = mybir.dt.bfloat16
F32 = mybir.dt.float32
AF = mybir.ActivationFunctionType
ALU = mybir.AluOpType


def build_bass(
    bias_on: str = "vector",
    score_bf16: bool = True,
    pipelined: bool = True,
    enc_bufs: int = 2,
    work_bufs: int = 3,
    pu_bufs: int = 4,
    pc_bufs: int = 2,
    wb_via: str = "gpsimd",
    reduce_on: str = "vector",
    dma_split: int = 4,
):
    nc = bacc.Bacc("TRN2", target_bir_lowering=False, debug=False)

    va_dt = BF if score_bf16 else F32
    th_dt = BF if score_bf16 else F32

    encT = nc.dram_tensor("encT", [BPC, D, T], BF, kind="ExternalInput")
    encN = nc.dram_tensor("encN", [BPC, T, D], BF, kind="ExternalInput")
    uawT = nc.dram_tensor("uawT", [D, H], BF, kind="ExternalInput")
    wawT = nc.dram_tensor("wawT", [D, H], BF, kind="ExternalInput")
    decT = nc.dram_tensor("decT", [D, BPC], BF, kind="ExternalInput")
    bsum = nc.dram_tensor("bsum", [1, H], BF, kind="ExternalInput")
    vabc = nc.dram_tensor("vabc", [P, H], va_dt, kind="ExternalInput")
    out = nc.dram_tensor("out", [BPC, D], F32, kind="ExternalOutput")

    with tile.TileContext(nc) as tc:
        with (
            tc.tile_pool(name="const", bufs=1) as cpool,
            tc.tile_pool(name="enc", bufs=enc_bufs) as epool,
            tc.tile_pool(name="work", bufs=work_bufs) as wpool,
            tc.tile_pool(name="pu", bufs=pu_bufs, space="PSUM") as pupool,
            tc.tile_pool(name="pc", bufs=pc_bufs, space="PSUM") as pcpool,
        ):
            # resident weights / constants
            UW = cpool.tile([P, DC, H], BF, tag="UW")
            nc.sync.dma_start(UW[:], uawT.ap().rearrange("(dc p) h -> p dc h", p=P))
            WW = cpool.tile([P, DC, H], BF, tag="WW")
            nc.sync.dma_start(WW[:], wawT.ap().rearrange("(dc p) h -> p dc h", p=P))
            DT = cpool.tile([P, DC, BPC], BF, tag="DT")
            nc.sync.dma_start(DT[:], decT.ap().rearrange("(dc p) b -> p dc b", p=P))
            BS = cpool.tile([1, H], BF, tag="BS")
            nc.sync.dma_start(BS[:], bsum.ap())
            VAB = cpool.tile([P, H], va_dt, tag="VAB")
            nc.sync.dma_start(VAB[:], vabc.ap())

            ones_r = cpool.tile([1, P], BF, tag="ones_r")
            nc.vector.memset(ones_r[:], 1.0)
            ones_c = cpool.tile([P, 1], BF, tag="ones_c")
            nc.vector.memset(ones_c[:], 1.0)

            # WaPB[b, h] = dec_b @ Wa_w.T + (Wa_b + Ua_b), all batches at once,
            # then flattened to one partition so per-b rows are base-0 matmul rhs.
            WaPBs = cpool.tile([BPC, H], BF, tag="WaPBs")
            for hh in range(2):
                pw = pcpool.tile([BPC, 512], F32, tag="pc")
                for dc in range(DC):
                    nc.tensor.matmul(
                        pw[:],
                        DT[:, dc, :],
                        WW[:, dc, ts(hh, 512)],
                        start=(dc == 0),
                        stop=False,
                    )
                nc.tensor.matmul(
                    pw[:],
                    ones_r[:, 0:BPC],
                    BS[:, ts(hh, 512)],
                    start=False,
                    stop=True,
                )
                nc.vector.tensor_copy(WaPBs[:, ts(hh, 512)], pw[:])
            WaPBrow = cpool.tile([1, BPC * H], BF, tag="WaPBrow")
            for b in range(BPC):
                nc.sync.dma_start(WaPBrow[:, b * H : (b + 1) * H], WaPBs[b : b + 1, :])

            def scores_stage(b):
                EB = epool.tile([P, DC, T], BF, tag="EB")
                srcT = encT.ap()[b].rearrange("(dc p) t -> p dc t", p=P)
                NB = epool.tile([P, TC, D], BF, tag="NB")
                srcN = encN.ap()[b].rearrange("(tc p) d -> p tc d", p=P)
                step = DC // dma_split
                for s in range(dma_split):
                    sl = slice(s * step, (s + 1) * step)
                    nc.sync.dma_start(EB[:, sl, :], srcT[:, sl, :])
                    nc.sync.dma_start(NB[:, sl, :], srcN[:, sl, :])

                WaPB = WaPBrow[:, b * H : (b + 1) * H]
                if bias_on == "vector":
                    # broadcast WaPB to 128 partitions once per b
                    if wb_via == "gpsimd":
                        WB = wpool.tile([P, H], BF, tag="WB")
                        nc.gpsimd.partition_broadcast(WB[:], WaPB)
                    else:
                        WB = wpool.tile([P, H], F32, tag="WB")
                        for hh in range(2):
                            pb = pcpool.tile([P, 512], F32, tag="pb")
                            nc.tensor.matmul(
                                pb[:],
                                ones_r[:],
                                WaPB[:, ts(hh, 512)],
                                start=True,
                                stop=True,
                            )
                            nc.vector.tensor_copy(WB[:, ts(hh, 512)], pb[:])
                SC = wpool.tile([P, TC], F32, tag="SC")
                for tci in range(TC):
                    pu0 = pupool.tile([P, 512], F32, tag="pu")
                    pu1 = pupool.tile([P, 512], F32, tag="pu")
                    last = bias_on != "tensor"
                    for dc in range(DC):
                        lh = EB[:, dc, ts(tci, P)]
                        nc.tensor.matmul(
                            pu0[:],
                            lh,
                            UW[:, dc, 0:512],
                            start=(dc == 0),
                            stop=(last and dc == DC - 1),
                        )
                        nc.tensor.matmul(
                            pu1[:],
                            lh,
                            UW[:, dc, 512:1024],
                            start=(dc == 0),
                            stop=(last and dc == DC - 1),
                        )
                    TH = wpool.tile([P, H], th_dt, tag="TH")
                    if bias_on == "tensor":
                        # += WaPB broadcast along t partitions (K=1 ones matmul)
                        nc.tensor.matmul(
                            pu0[:], ones_r[:], WaPB[:, 0:512], start=False, stop=True
                        )
                        nc.tensor.matmul(
                            pu1[:], ones_r[:], WaPB[:, 512:1024], start=False, stop=True
                        )
                        nc.scalar.activation(TH[:, 0:512], pu0[:], AF.Tanh)
                        nc.scalar.activation(TH[:, 512:1024], pu1[:], AF.Tanh)
                    else:
                        T1 = wpool.tile([P, H], F32, tag="T1")
                        nc.vector.tensor_tensor(
                            T1[:, 0:512], pu0[:], WB[:, 0:512], ALU.add
                        )
                        nc.vector.tensor_tensor(
                            T1[:, 512:1024], pu1[:], WB[:, 512:1024], ALU.add
                        )
                        nc.scalar.activation(TH[:, 0:512], T1[:, 0:512], AF.Tanh)
                        nc.scalar.activation(TH[:, 512:1024], T1[:, 512:1024], AF.Tanh)
                    TMP = wpool.tile([P, H], th_dt, tag="TMP")
                    nc.vector.tensor_tensor(TMP[:], TH[:], VAB[:], ALU.mult)
                    if reduce_on == "scalar":
                        TJ = wpool.tile([P, H], th_dt, tag="TJ")
                        nc.scalar.activation(
                            TJ[:],
                            TMP[:],
                            AF.Identity,
                            accum_out=SC[:, tci : tci + 1],
                        )
                    else:
                        nc.vector.tensor_reduce(
                            SC[:, tci : tci + 1],
                            TMP[:],
                            axis=mybir.AxisListType.X,
                            op=ALU.add,
                        )
                return SC, NB

            def ctx_stage(b, SC, NB):
                # unnormalized softmax weights, bf16 columns [128t, TC]
                EW = wpool.tile([P, TC], BF, tag="EW")
                nc.scalar.activation(EW[:], SC[:], AF.Exp)
                psum_s = pcpool.tile([1, TC], F32, tag="pc")
                nc.tensor.matmul(psum_s[:], ones_c[:], EW[:], start=True, stop=True)
                TOT = wpool.tile([1, 1], F32, tag="TOT")
                nc.vector.tensor_reduce(
                    TOT[:], psum_s[:], axis=mybir.AxisListType.X, op=ALU.add
                )
                INV = wpool.tile([1, 1], F32, tag="INV")
                nc.vector.reciprocal(INV[:], TOT[:])

                OUTb = wpool.tile([1, D], F32, tag="OUTb")
                for dh in range(2):
                    pc = pcpool.tile([1, 512], F32, tag="pc")
                    for tci in range(TC):
                        nc.tensor.matmul(
                            pc[:],
                            EW[:, tci : tci + 1],
                            NB[:, tci, ts(dh, 512)],
                            start=(tci == 0),
                            stop=(tci == TC - 1),
                        )
                    nc.scalar.activation(
                        OUTb[:, ts(dh, 512)], pc[:], AF.Copy, scale=INV[:]
                    )
                nc.sync.dma_start(out.ap()[b : b + 1, :], OUTb[:])

            if pipelined:
                prev = None
                for b in range(BPC):
                    cur = scores_stage(b)
                    if prev is not None:
                        ctx_stage(b - 1, *prev)
                    prev = cur
                ctx_stage(BPC - 1, *prev)
            else:
                for b in range(BPC):
                    SC, NB = scores_stage(b)
                    ctx_stage(b, SC, NB)

    nc.finalize()
    return nc


_NC = None


def _get_nc():
    global _NC
    if _NC is None:
        _NC = build_bass()
    return _NC


LAST_RESULTS = None


def kernel(**inputs) -> np.ndarray:
    enc = np.asarray(inputs["encoder_outputs"], dtype=np.float32)  # [B, T, D]
    dec = np.asarray(inputs["decoder_outputs"], dtype=np.float32)[:, 0, :]  # [B, D]
    Wa_w = np.asarray(inputs["Wa_w"], dtype=np.float32)
    Wa_b = np.asarray(inputs["Wa_b"], dtype=np.float32)
    Ua_w = np.asarray(inputs["Ua_w"], dtype=np.float32)
    Ua_b = np.asarray(inputs["Ua_b"], dtype=np.float32)
    Va_w = np.asarray(inputs["Va_w"], dtype=np.float32)
    # Va_b dropped: softmax(s + c) == softmax(s)

    bf16 = ml_dtypes.bfloat16
    enc_bf = enc.astype(bf16)  # [B, T, D]
    encN_all = enc_bf.reshape(NCORES, BPC, T, D)
    encT_all = np.ascontiguousarray(enc_bf.transpose(0, 2, 1)).reshape(
        NCORES, BPC, D, T
    )
    decT_all = np.ascontiguousarray(
        dec.reshape(NCORES, BPC, D).transpose(0, 2, 1)
    ).astype(bf16)  # [NCORES, D, BPC]
    uawT = np.ascontiguousarray(Ua_w.T).astype(bf16)
    wawT = np.ascontiguousarray(Wa_w.T).astype(bf16)
    bsum = (Wa_b + Ua_b).reshape(1, H).astype(bf16)
    vabc = np.ascontiguousarray(np.broadcast_to(Va_w.reshape(1, H), (P, H))).astype(
        bf16
    )

    in_maps = [
        {
            "encT": np.ascontiguousarray(encT_all[c]),
            "encN": np.ascontiguousarray(encN_all[c]),
            "uawT": uawT,
            "wawT": wawT,
            "decT": np.ascontiguousarray(decT_all[c]),
            "bsum": bsum,
            "vabc": vabc,
        }
        for c in range(NCORES)
    ]

    nc = _get_nc()
    trace = bool(int(os.environ.get("KERNEL_TRACE", "0")))
    res = run_bass_kernel_spmd(nc, in_maps, core_ids=list(range(NCORES)), trace=trace)
    global LAST_RESULTS
    LAST_RESULTS = res

    outs = [res.results[c]["out"] for c in range(NCORES)]
    full = np.concatenate(outs, axis=0).reshape(B, 1, D).astype(np.float32)
    return full


# revision 18
# speedup vs baseline: 69054.4533x; 19233.3827x over previous
"""Bahdanau additive attention kernel for 8 Trainium2 NeuronCores.

Data-parallel over batch: B=64 -> 8 batches per core. No collectives.

Per-batch math (reference):
  Wa   = dec @ Wa_w.T + Wa_b                       [1, H]
  Ua   = enc @ Ua_w.T + Ua_b                       [Te, H]
  s    = tanh(Ua + Wa) @ Va_w.T  (+ Va_b, dropped: softmax shift-invariant)
  w    = softmax(s)                                 [Te]
  ctx  = w @ enc                                    [1, De]

Device layout (per core, 8 batches):
  big matmul Ua:  out[t, h] = sum_d encT[d, t] * uawT[d, h]    (bf16, fp32 PSUM)
  bias:           VectorE add of WaPB broadcast (built once per b via ones mm)
  tanh:           ScalarE, SBUF -> SBUF bf16
  score:          VectorE mult by Va bcast (bf16 2x mode) + reduce over h
                  -> scores as columns [128t, 8 chunks]
  softmax:        exp on ScalarE (scores bounded, no max subtraction),
                  normalization deferred to the end (unnormalized weights)
  context:        matmul, exp-weight column stationary vs encN [t, d] moving;
                  software-pipelined one batch behind the score stage
  scale:          ScalarE copy with scale = 1/sum(exp)
"""

import os
import sys

import numpy as np
import ml_dtypes

for _p in ("/opt/trn_rl_repo",):
    if _p not in sys.path and os.path.isdir(_p):
        sys.path.append(_p)

import concourse.bass as bass
import concourse.tile as tile
import concourse.mybir as mybir
from concourse import bacc
from concourse.bass import ts
from concourse.bass_utils import run_bass_kernel_spmd

B, T, D, H = 64, 1024, 1024, 1024
NCORES = 8
BPC = B // NCORES  # batches per core
P = 128
DC = D // P  # 8 contraction chunks
TC = T // P  # 8 t chunks

BF = mybir.dt.bfloat16
F32 = mybir.dt.float32
AF = mybir.ActivationFunctionType
ALU = mybir.AluOpType


def build_bass(
    bias_on: str = "vector",
    score_bf16: bool = True,
    pipelined: bool = True,
    enc_bufs: int = 2,
    work_bufs: int = 3,
    pu_bufs: int = 4,
    pc_bufs: int = 2,
    wb_via: str = "gpsimd",
    reduce_on: str = "vector",
    dma_split: int = 1,
):
    nc = bacc.Bacc("TRN2", target_bir_lowering=False, debug=False)

    va_dt = BF if score_bf16 else F32
    th_dt = BF if score_bf16 else F32

    encT = nc.dram_tensor("encT", [BPC, D, T], BF, kind="ExternalInput")
    encN = nc.dram_tensor("encN", [BPC, T, D], BF, kind="ExternalInput")
    uawT = nc.dram_tensor("uawT", [D, H], BF, kind="ExternalInput")
    wawT = nc.dram_tensor("wawT", [D, H], BF, kind="ExternalInput")
    decT = nc.dram_tensor("decT", [D, BPC], BF, kind="ExternalInput")
    bsum = nc.dram_tensor("bsum", [1, H], BF, kind="ExternalInput")
    vabc = nc.dram_tensor("vabc", [P, H], va_dt, kind="ExternalInput")
    out = nc.dram_tensor("out", [BPC, D], F32, kind="ExternalOutput")

    with tile.TileContext(nc) as tc:
        with (
            tc.tile_pool(name="const", bufs=1) as cpool,
            tc.tile_pool(name="enc", bufs=enc_bufs) as epool,
            tc.tile_pool(name="work", bufs=work_bufs) as wpool,
            tc.tile_pool(name="pu", bufs=pu_bufs, space="PSUM") as pupool,
            tc.tile_pool(name="pc", bufs=pc_bufs, space="PSUM") as pcpool,
        ):
            # resident weights / constants
            UW = cpool.tile([P, DC, H], BF, tag="UW")
            nc.sync.dma_start(UW[:], uawT.ap().rearrange("(dc p) h -> p dc h", p=P))
            WW = cpool.tile([P, DC, H], BF, tag="WW")
            nc.sync.dma_start(WW[:], wawT.ap().rearrange("(dc p) h -> p dc h", p=P))
            DT = cpool.tile([P, DC, BPC], BF, tag="DT")
            nc.sync.dma_start(DT[:], decT.ap().rearrange("(dc p) b -> p dc b", p=P))
            BS = cpool.tile([1, H], BF, tag="BS")
            nc.sync.dma_start(BS[:], bsum.ap())
            VAB = cpool.tile([P, H], va_dt, tag="VAB")
            nc.sync.dma_start(VAB[:], vabc.ap())

            ones_r = cpool.tile([1, P], BF, tag="ones_r")
            nc.vector.memset(ones_r[:], 1.0)
            ones_c = cpool.tile([P, 1], BF, tag="ones_c")
            nc.vector.memset(ones_c[:], 1.0)

            # WaPB[b, h] = dec_b @ Wa_w.T + (Wa_b + Ua_b), all batches at once,
            # then flattened to one partition so per-b rows are base-0 matmul rhs.
            WaPBs = cpool.tile([BPC, H], BF, tag="WaPBs")
            for hh in range(2):
                pw = pcpool.tile([BPC, 512], F32, tag="pc")
                for dc in range(DC):
                    nc.tensor.matmul(
                        pw[:],
                        DT[:, dc, :],
                        WW[:, dc, ts(hh, 512)],
                        start=(dc == 0),
                        stop=False,
                    )
                nc.tensor.matmul(
                    pw[:],
                    ones_r[:, 0:BPC],
                    BS[:, ts(hh, 512)],
                    start=False,
                    stop=True,
                )
                nc.vector.tensor_copy(WaPBs[:, ts(hh, 512)], pw[:])
            WaPBrow = cpool.tile([1, BPC * H], BF, tag="WaPBrow")
            for b in range(BPC):
                nc.sync.dma_start(WaPBrow[:, b * H : (b + 1) * H], WaPBs[b : b + 1, :])

            def scores_stage(b):
                EB = epool.tile([P, DC, T], BF, tag="EB")
                srcT = encT.ap()[b].rearrange("(dc p) t -> p dc t", p=P)
                NB = epool.tile([P, TC, D], BF, tag="NB")
                srcN = encN.ap()[b].rearrange("(tc p) d -> p tc d", p=P)
                step = DC // dma_split
                for s in range(dma_split):
                    sl = slice(s * step, (s + 1) * step)
                    nc.sync.dma_start(EB[:, sl, :], srcT[:, sl, :])
                    nc.sync.dma_start(NB[:, sl, :], srcN[:, sl, :])

                WaPB = WaPBrow[:, b * H : (b + 1) * H]
                if bias_on == "vector":
                    # broadcast WaPB to 128 partitions once per b
                    if wb_via == "gpsimd":
                        WB = wpool.tile([P, H], BF, tag="WB")
                        nc.gpsimd.partition_broadcast(WB[:], WaPB)
                    else:
                        WB = wpool.tile([P, H], F32, tag="WB")
                        for hh in range(2):
                            pb = pcpool.tile([P, 512], F32, tag="pb")
                            nc.tensor.matmul(
                                pb[:],
                                ones_r[:],
                                WaPB[:, ts(hh, 512)],
                                start=True,
                                stop=True,
                            )
                            nc.vector.tensor_copy(WB[:, ts(hh, 512)], pb[:])
                SC = wpool.tile([P, TC], F32, tag="SC")
                for tci in range(TC):
                    pu0 = pupool.tile([P, 512], F32, tag="pu")
                    pu1 = pupool.tile([P, 512], F32, tag="pu")
                    last = bias_on != "tensor"
                    for dc in range(DC):
                        lh = EB[:, dc, ts(tci, P)]
                        nc.tensor.matmul(
                            pu0[:],
                            lh,
                            UW[:, dc, 0:512],
                            start=(dc == 0),
                            stop=(last and dc == DC - 1),
                        )
                        nc.tensor.matmul(
                            pu1[:],
                            lh,
                            UW[:, dc, 512:1024],
                            start=(dc == 0),
                            stop=(last and dc == DC - 1),
                        )
                    TH = wpool.tile([P, H], th_dt, tag="TH")
                    if bias_on == "tensor":
                        # += WaPB broadcast along t partitions (K=1 ones matmul)
                        nc.tensor.matmul(
                            pu0[:], ones_r[:], WaPB[:, 0:512], start=False, stop=True
                        )
                        nc.tensor.matmul(
                            pu1[:], ones_r[:], WaPB[:, 512:1024], start=False, stop=True
                        )
                        nc.scalar.activation(TH[:, 0:512], pu0[:], AF.Tanh)
                        nc.scalar.activation(TH[:, 512:1024], pu1[:], AF.Tanh)
                    else:
                        T1 = wpool.tile([P, H], F32, tag="T1")
                        nc.vector.tensor_tensor(
                            T1[:, 0:512], pu0[:], WB[:, 0:512], ALU.add
                        )
                        nc.vector.tensor_tensor(
                            T1[:, 512:1024], pu1[:], WB[:, 512:1024], ALU.add
                        )
                        nc.scalar.activation(TH[:, 0:512], T1[:, 0:512], AF.Tanh)
                        nc.scalar.activation(TH[:, 512:1024], T1[:, 512:1024], AF.Tanh)
                    TMP = wpool.tile([P, H], th_dt, tag="TMP")
                    nc.vector.tensor_tensor(TMP[:], TH[:], VAB[:], ALU.mult)
                    if reduce_on == "scalar":
                        TJ = wpool.tile([P, H], th_dt, tag="TJ")
                        nc.scalar.activation(
                            TJ[:],
                            TMP[:],
                            AF.Identity,
                            accum_out=SC[:, tci : tci + 1],
                        )
                    else:
                        nc.vector.tensor_reduce(
                            SC[:, tci : tci + 1],
                            TMP[:],
                            axis=mybir.AxisListType.X,
                            op=ALU.add,
                        )
                return SC, NB

            def ctx_stage(b, SC, NB):
                # unnormalized softmax weights, bf16 columns [128t, TC]
                EW = wpool.tile([P, TC], BF, tag="EW")
                nc.scalar.activation(EW[:], SC[:], AF.Exp)
                psum_s = pcpool.tile([1, TC], F32, tag="pc")
                nc.tensor.matmul(psum_s[:], ones_c[:], EW[:], start=True, stop=True)
                TOT = wpool.tile([1, 1], F32, tag="TOT")
                nc.vector.tensor_reduce(
                    TOT[:], psum_s[:], axis=mybir.AxisListType.X, op=ALU.add
                )
                INV = wpool.tile([1, 1], F32, tag="INV")
                nc.vector.reciprocal(INV[:], TOT[:])

                OUTb = wpool.tile([1, D], F32, tag="OUTb")
                for dh in range(2):
                    pc = pcpool.tile([1, 512], F32, tag="pc")
                    for tci in range(TC):
                        nc.tensor.matmul(
                            pc[:],
                            EW[:, tci : tci + 1],
                            NB[:, tci, ts(dh, 512)],
                            start=(tci == 0),
                            stop=(tci == TC - 1),
                        )
                    nc.scalar.activation(
                        OUTb[:, ts(dh, 512)], pc[:], AF.Copy, scale=INV[:]
                    )
                nc.sync.dma_start(out.ap()[b : b + 1, :], OUTb[:])

            if pipelined:
                prev = None
                for b in range(BPC):
                    cur = scores_stage(b)
                    if prev is not None:
                        ctx_stage(b - 1, *prev)
                    prev = cur
                ctx_stage(BPC - 1, *prev)
            else:
                for b in range(BPC):
                    SC, NB = scores_stage(b)
                    ctx_stage(b, SC, NB)

    nc.finalize()
    return nc


_NC = None


def _get_nc():
    global _NC
    if _NC is None:
        _NC = build_bass()
    return _NC


LAST_RESULTS = None


def kernel(**inputs) -> np.ndarray:
    enc = np.asarray(inputs["encoder_outputs"], dtype=np.float32)  # [B, T, D]
    dec = np.asarray(inputs["decoder_outputs"], dtype=np.float32)[:, 0, :]  # [B, D]
    Wa_w = np.asarray(inputs["Wa_w"], dtype=np.float32)
    Wa_b = np.asarray(inputs["Wa_b"], dtype=np.float32)
    Ua_w = np.asarray(inputs["Ua_w"], dtype=np.float32)
    Ua_b = np.asarray(inputs["Ua_b"], dtype=np.float32)
    Va_w = np.asarray(inputs["Va_w"], dtype=np.float32)
    # Va_b dropped: softmax(s + c) == softmax(s)

    bf16 = ml_dtypes.bfloat16
    enc_bf = enc.astype(bf16)  # [B, T, D]
    encN_all = enc_bf.reshape(NCORES, BPC, T, D)
    encT_all = np.ascontiguousarray(enc_bf.transpose(0, 2, 1)).reshape(
        NCORES, BPC, D, T
    )
    decT_all = np.ascontiguousarray(
        dec.reshape(NCORES, BPC, D).transpose(0, 2, 1)
    ).astype(bf16)  # [NCORES, D, BPC]
    uawT = np.ascontiguousarray(Ua_w.T).astype(bf16)
    wawT = np.ascontiguousarray(Wa_w.T).astype(bf16)
    bsum = (Wa_b + Ua_b).reshape(1, H).astype(bf16)
    vabc = np.ascontiguousarray(np.broadcast_to(Va_w.reshape(1, H), (P, H))).astype(
        bf16
    )

    in_maps = [
        {
            "encT": np.ascontiguousarray(encT_all[c]),
            "encN": np.ascontiguousarray(encN_all[c]),
            "uawT": uawT,
            "wawT": wawT,
            "decT": np.ascontiguousarray(decT_all[c]),
            "bsum": bsum,
            "vabc": vabc,
        }
        for c in range(NCORES)
    ]

    nc = _get_nc()
    trace = bool(int(os.environ.get("KERNEL_TRACE", "0")))
    res = run_bass_kernel_spmd(nc, in_maps, core_ids=list(range(NCORES)), trace=trace)
    global LAST_RESULTS
    LAST_RESULTS = res

    outs = [res.results[c]["out"] for c in range(NCORES)]
    full = np.concatenate(outs, axis=0).reshape(B, 1, D).astype(np.float32)
    return full
